# revision 6
# baseline (speedup 1.0000x reference)
"""Contrastive-learning loss kernel for 8 TRN2 NeuronCores.

loss = logsumexp(sim_neg / T) - mean(sim_pos) / T

where sim_pos/sim_neg are all-pairs cosine similarities. Two reductions:
  denom = sum_ij exp(s_i . r_j / T)        (needs the full N x N matmul)
  sum(sim_pos) = (sum_i s_i) . (sum_j b_j) (rank-1 identity, no matmul)
with s/r/b = row-normalized x_source / x_raw_target / x_bc_target.

Sharding (2 x 4 grid over the sim_neg matrix): core c = a*4+b gets
S rows [a*2048, (a+1)*2048) and R rows [b*1024, (b+1)*1024), plus a
distinct 512-row shard of x_bc_target and x_source for the numerator
partials. Each core returns partial exp-sums and weighted row-sums;
the host combines the 8 partials in float64 and takes the log.
"""

import json

import numpy as np

import concourse.bass as bass
import concourse.mybir as mybir
import concourse.tile as tile
from concourse.bass_utils import run_bass_kernel_spmd
from concourse.masks import make_identity
from concourse.vector_clock import ScopedClock, VectorClock

P = 128
N = 4096
D = 2048
TEMP = 0.5
A_SPLIT = 2  # S-row blocks
B_SPLIT = 4  # R-row blocks
SB = N // A_SPLIT  # 2048 source rows per core
RB = N // B_SPLIT  # 1024 raw-target rows per core
NSH = N // 8  # 512 numerator-shard rows per core
KT = D // P  # 16 contraction tiles
ST = SB // P  # 16 source row-tiles per core
RT = RB // P  # 8 raw row-tiles per core
JB = RB // 512  # 2 psum-bank columns of the sim block

F32 = mybir.dt.float32
F32R = mybir.dt.float32r
AF = mybir.ActivationFunctionType


def _spill_sync_waits(raw: bytes) -> bytes:
    """The walrus here has no sync-wait slots on Matmult (fused weight-load
    S3_LW struct) and chokes on multi-wait instructions generally. Move every
    Matmult wait — and all but the first wait of any other instruction — onto
    single-wait NoOps inserted just before it on the same engine queue."""
    d = json.loads(raw)
    ctr = 0
    for fn in d["functions"]:
        for blk in fn["blocks"]:
            out = []
            for inst in blk["instructions"]:
                si = inst.get("sync_info")
                waits = si.get("on_wait") if si else None
                limit = 0 if inst.get("opcode") == "Matmult" else 1
                if waits and len(waits) > limit:
                    for w in waits[limit:]:
                        ctr += 1
                        out.append(
                            {
                                "debug": inst.get("debug"),
                                "engine": inst["engine"],
                                "ins": [],
                                "name": f"I-waitfix-{ctr}",
                                "opcode": "NoOp",
                                "outs": [],
                                "sync_info": {"on_update": [], "on_wait": [w]},
                            }
                        )
                    si["on_wait"] = waits[:limit]
                out.append(inst)
            blk["instructions"] = out
    return json.dumps(d).encode()


class PatchedBass(bass.Bass):
    def to_json_bytes(self) -> bytes:
        return _spill_sync_waits(super().to_json_bytes())


class TC(tile.TileContext):
    """TileContext whose kernel-tail drain carries its sem waits on
    single-wait NOPs — this walrus rejects multi-wait Drain instructions."""

    def _drain_and_barrier(self, tick_clock, wait_clock):
        g = tick_clock.global_clock
        nprocs = len(g)
        for p in range(nprocs):
            t = g[p]
            if t <= 0:
                continue
            vec = [0] * nprocs
            vec[p] = t
            nop = self.nc.sync.nop(nofuse=True)
            wait_clock.add_sem_waits(nop.ins, ScopedClock({None: VectorClock(vec)}))
        self.nc.sync.drain()
        self.nc.all_engine_barrier()
        assert self.sems is not None
        popped = self.nc._tile_sem_poison_stack.pop()
        assert popped is self._sem_poison
        self.nc.clear_and_free_semaphores(list(self.sems.allocated().values()))
        self.nc.all_engine_barrier()


def build():
    nc = PatchedBass()
    s_block = nc.dram_tensor("s_block", [SB, D], F32, kind="ExternalInput")
    r_block = nc.dram_tensor("r_block", [RB, D], F32, kind="ExternalInput")
    b_shard = nc.dram_tensor("b_shard", [NSH, D], F32, kind="ExternalInput")
    sn_shard = nc.dram_tensor("sn_shard", [NSH, D], F32, kind="ExternalInput")
    denom_acc = nc.dram_tensor("denom_acc", [P, ST * JB], F32, kind="ExternalOutput")
    ssum = nc.dram_tensor("ssum", [1, D], F32, kind="ExternalOutput")
    bsum = nc.dram_tensor("bsum", [1, D], F32, kind="ExternalOutput")

    with TC(nc) as tc:
        with (
            tc.tile_pool(name="big", bufs=1) as big,
            tc.tile_pool(name="xin", bufs=3) as xin,
            tc.tile_pool(name="sqp", bufs=2) as sqp,
            tc.tile_pool(name="stp", bufs=2) as stp,
            tc.tile_pool(name="escp", bufs=2) as escp,
            tc.tile_pool(name="vecp", bufs=3) as vecp,
            tc.tile_pool(name="tpsum", bufs=2, space="PSUM") as tpsum,
            tc.tile_pool(name="gpsum", bufs=2, space="PSUM") as gpsum,
            tc.tile_pool(name="vpsum", bufs=1, space="PSUM") as vpsum,
        ):
            ident = big.tile([P, P], F32, name="ident")
            make_identity(nc, ident)
            rT = big.tile([P, KT, RB], F32R, name="rT")
            dacc = big.tile([P, ST * JB], F32, name="dacc")

            def inv_norm(x):
                """Per-row 1/||x_row|| for a [128, D] tile."""
                sq = sqp.tile([P, D], F32, tag="sq", name="sq")
                ssq = vecp.tile([P, 1], F32, tag="ssq", name="ssq")
                nc.scalar.activation(out=sq, in_=x, func=AF.Square, accum_out=ssq)
                nrm = vecp.tile([P, 1], F32, tag="nrm", name="nrm")
                nc.scalar.sqrt(nrm, ssq)
                inv = vecp.tile([P, 1], F32, tag="inv", name="inv")
                nc.vector.reciprocal(out=inv, in_=nrm)
                return inv

            def transpose_tile(x, dst):
                """PE-transpose a [128, D] tile into dst [128, KT, 128]."""
                for kb in range(KT // 4):
                    tp = tpsum.tile([P, 512], F32, tag="tp", name="tp")
                    for q in range(4):
                        k = kb * 4 + q
                        nc.tensor.transpose(
                            tp[:, q * P : (q + 1) * P],
                            x[:, k * P : (k + 1) * P],
                            ident,
                        )
                    nc.vector.tensor_copy(
                        out=dst[:, kb * 4 : (kb + 1) * 4, :],
                        in_=tp.rearrange("p (a b) -> p a b", a=4),
                    )

            def numerator(src, out_dram, label):
                """sum over shard rows of row/||row|| -> out_dram [1, D]."""
                chunks = [
                    vpsum.tile([1, 512], F32, tag=f"vp{cc}", name=f"vp{cc}_{label}")
                    for cc in range(4)
                ]
                nt = NSH // P
                for t in range(nt):
                    x = xin.tile([P, D], F32, tag="xin", name=f"nx_{label}")
                    nc.sync.dma_start(out=x, in_=src[t * P : (t + 1) * P, :])
                    inv = inv_norm(x)
                    for cc in range(4):
                        nc.tensor.matmul(
                            chunks[cc],
                            lhsT=inv,
                            rhs=x[:, cc * 512 : (cc + 1) * 512],
                            start=(t == 0),
                            stop=(t == nt - 1),
                        )
                osb = vecp.tile([1, D], F32, tag="osb", name=f"osb_{label}")
                for cc in range(4):
                    nc.vector.tensor_copy(
                        out=osb[:, cc * 512 : (cc + 1) * 512], in_=chunks[cc]
                    )
                nc.sync.dma_start(out=out_dram[:, :], in_=osb)

            numerator(b_shard, bsum, "b")
            numerator(sn_shard, ssum, "s")

            # ---- R block: normalize rows, transpose into rT [d, j] layout
            for jt in range(RT):
                rx = xin.tile([P, D], F32, tag="xin", name="rx")
                nc.sync.dma_start(out=rx, in_=r_block[jt * P : (jt + 1) * P, :])
                inv = inv_norm(rx)
                rxn = xin.tile([P, D], F32, tag="xn", bufs=2, name="rxn")
                nc.scalar.mul(rxn, rx, inv)
                transpose_tile(rxn, rT[:, :, jt * P : (jt + 1) * P])

            # ---- S loop: raw transpose, matmul vs rT, exp+reduce
            for st in range(ST):
                sx = xin.tile([P, D], F32, tag="xin", name="sx")
                nc.sync.dma_start(out=sx, in_=s_block[st * P : (st + 1) * P, :])
                inv_s = inv_norm(sx)
                inv_sT = vecp.tile([P, 1], F32, tag="invT", name="invT")
                nc.scalar.mul(inv_sT, inv_s, 1.0 / TEMP)
                sT = stp.tile([P, KT, P], F32R, tag="sT", name="sT")
                transpose_tile(sx, sT)
                for jb in range(JB):
                    g = gpsum.tile([P, 512], F32, tag="g", name="g")
                    for k in range(KT):
                        nc.tensor.matmul(
                            g,
                            lhsT=sT[:, k, :],
                            rhs=rT[:, k, jb * 512 : (jb + 1) * 512],
                            start=(k == 0),
                            stop=(k == KT - 1),
                        )
                    esc = escp.tile([P, 512], F32, tag="esc", name="esc")
                    col = st * JB + jb
                    nc.scalar.activation(
                        out=esc,
                        in_=g,
                        func=AF.Exp,
                        scale=inv_sT,
                        accum_out=dacc[:, col : col + 1],
                    )

            nc.sync.dma_start(out=denom_acc[:, :], in_=dacc)
    return nc


_NC_CACHE = {}


def _get_nc():
    if "nc" not in _NC_CACHE:
        _NC_CACHE["nc"] = build()
    return _NC_CACHE["nc"]


def _make_in_maps(x_source, x_bc_target, x_raw_target):
    in_maps = []
    for c in range(8):
        a, b = c // B_SPLIT, c % B_SPLIT
        in_maps.append(
            {
                "s_block": np.ascontiguousarray(
                    x_source[a * SB : (a + 1) * SB], dtype=np.float32
                ),
                "r_block": np.ascontiguousarray(
                    x_raw_target[b * RB : (b + 1) * RB], dtype=np.float32
                ),
                "b_shard": np.ascontiguousarray(
                    x_bc_target[c * NSH : (c + 1) * NSH], dtype=np.float32
                ),
                "sn_shard": np.ascontiguousarray(
                    x_source[c * NSH : (c + 1) * NSH], dtype=np.float32
                ),
            }
        )
    return in_maps


def _combine(results):
    denom = 0.0
    s_tot = np.zeros(D, dtype=np.float64)
    b_tot = np.zeros(D, dtype=np.float64)
    for r in results:
        denom += r["denom_acc"].astype(np.float64).sum()
        s_tot += r["ssum"][0].astype(np.float64)
        b_tot += r["bsum"][0].astype(np.float64)
    loss = np.log(denom) - (s_tot @ b_tot) / (float(N) * float(N)) / TEMP
    return np.array(loss, dtype=np.float32)


def _run(x_source, x_bc_target, x_raw_target, trace=False):
    nc = _get_nc()
    in_maps = _make_in_maps(x_source, x_bc_target, x_raw_target)
    res = run_bass_kernel_spmd(nc, in_maps, core_ids=list(range(8)), trace=trace)
    return _combine(res.results), res


def kernel(x_source, x_bc_target, x_raw_target):
    out, _ = _run(x_source, x_bc_target, x_raw_target)
    return out


# revision 8
# speedup vs baseline: 1.1193x; 1.1193x over previous
"""Contrastive-learning loss kernel for 8 TRN2 NeuronCores.

loss = logsumexp(sim_neg / T) - mean(sim_pos) / T

where sim_pos/sim_neg are all-pairs cosine similarities. Two reductions:
  denom = sum_ij exp(s_i . r_j / T)        (needs the full N x N matmul)
  sum(sim_pos) = (sum_i s_i) . (sum_j b_j) (rank-1 identity, no matmul)
with s/r/b = row-normalized x_source / x_raw_target / x_bc_target.

Sharding (2 x 4 grid over the sim_neg matrix): core c = a*4+b gets
S rows [a*2048, (a+1)*2048) and R rows [b*1024, (b+1)*1024), plus a
distinct 512-row shard of x_bc_target for the numerator partial (the
x_source numerator shard is rows b*512.. of its own S block). Each
core returns partial exp-sums and inv-norm-weighted row-sums; the host
combines the 8 partials in float64 and takes the log.

All matmul traffic is float32r (tf32-like PE fast path, 1 cyc/row at
N>=256; transposes 1.5 cyc/row). The BIR verifier requires f32r matmul
operands to come from f32r-producing instructions, so the DRAM inputs
and every tile on the matmul path are declared f32r; ACT/DVE ops that
just read values use .bitcast(f32) views.
"""

import json

import numpy as np

import concourse.bass as bass
import concourse.mybir as mybir
import concourse.tile as tile
from concourse.bass_utils import run_bass_kernel_spmd
from concourse.masks import make_identity
from concourse.vector_clock import ScopedClock, VectorClock

P = 128
N = 4096
D = 2048
TEMP = 0.5
A_SPLIT = 2  # S-row blocks
B_SPLIT = 4  # R-row blocks
SB = N // A_SPLIT  # 2048 source rows per core
RB = N // B_SPLIT  # 1024 raw-target rows per core
NSH = N // 8  # 512 numerator-shard rows per core
KT = D // P  # 16 contraction tiles
ST = SB // P  # 16 source row-tiles per core
RT = RB // P  # 8 raw row-tiles per core
JB = RB // 512  # 2 psum-bank columns of the sim block

F32 = mybir.dt.float32
F32R = mybir.dt.float32r
AF = mybir.ActivationFunctionType


def _spill_sync_waits(raw: bytes) -> bytes:
    """The walrus here has no sync-wait slots on Matmult (fused weight-load
    S3_LW struct) and chokes on multi-wait instructions generally. Move every
    Matmult wait — and all but the first wait of any other instruction — onto
    single-wait NoOps inserted just before it on the same engine queue."""
    d = json.loads(raw)
    ctr = 0
    for fn in d["functions"]:
        for blk in fn["blocks"]:
            out = []
            for inst in blk["instructions"]:
                si = inst.get("sync_info")
                waits = si.get("on_wait") if si else None
                limit = 0 if inst.get("opcode") == "Matmult" else 1
                if waits and len(waits) > limit:
                    for w in waits[limit:]:
                        ctr += 1
                        out.append(
                            {
                                "debug": inst.get("debug"),
                                "engine": inst["engine"],
                                "ins": [],
                                "name": f"I-waitfix-{ctr}",
                                "opcode": "NoOp",
                                "outs": [],
                                "sync_info": {"on_update": [], "on_wait": [w]},
                            }
                        )
                    si["on_wait"] = waits[:limit]
                out.append(inst)
            blk["instructions"] = out
    return json.dumps(d).encode()


class PatchedBass(bass.Bass):
    def to_json_bytes(self) -> bytes:
        return _spill_sync_waits(super().to_json_bytes())


class TC(tile.TileContext):
    """TileContext whose kernel-tail drain carries its sem waits on
    single-wait NOPs — this walrus rejects multi-wait Drain instructions."""

    def _drain_and_barrier(self, tick_clock, wait_clock):
        g = tick_clock.global_clock
        nprocs = len(g)
        for p in range(nprocs):
            t = g[p]
            if t <= 0:
                continue
            vec = [0] * nprocs
            vec[p] = t
            nop = self.nc.sync.nop(nofuse=True)
            wait_clock.add_sem_waits(nop.ins, ScopedClock({None: VectorClock(vec)}))
        self.nc.sync.drain()
        self.nc.all_engine_barrier()
        assert self.sems is not None
        popped = self.nc._tile_sem_poison_stack.pop()
        assert popped is self._sem_poison
        self.nc.clear_and_free_semaphores(list(self.sems.allocated().values()))
        self.nc.all_engine_barrier()


def build():
    nc = PatchedBass()
    s_block = nc.dram_tensor("s_block", [SB, D], F32R, kind="ExternalInput")
    r_block = nc.dram_tensor("r_block", [RB, D], F32R, kind="ExternalInput")
    b_shard = nc.dram_tensor("b_shard", [NSH, D], F32R, kind="ExternalInput")
    denom_acc = nc.dram_tensor("denom_acc", [P, ST * JB], F32, kind="ExternalOutput")
    ssum = nc.dram_tensor("ssum", [1, D], F32, kind="ExternalOutput")
    bsum = nc.dram_tensor("bsum", [1, D], F32, kind="ExternalOutput")

    with TC(nc) as tc:
        with (
            tc.tile_pool(name="big", bufs=1) as big,
            tc.tile_pool(name="xin", bufs=3) as xin,
            tc.tile_pool(name="sqp", bufs=2) as sqp,
            tc.tile_pool(name="stp", bufs=2) as stp,
            tc.tile_pool(name="escp", bufs=2) as escp,
            tc.tile_pool(name="vecp", bufs=3) as vecp,
            tc.tile_pool(name="tpsum", bufs=2, space="PSUM") as tpsum,
            tc.tile_pool(name="gpsum", bufs=2, space="PSUM") as gpsum,
            tc.tile_pool(name="vpsum", bufs=1, space="PSUM") as vpsum,
        ):
            identF = big.tile([P, P], F32, name="identF")
            make_identity(nc, identF)
            ident = big.tile([P, P], F32R, name="ident")
            nc.vector.tensor_copy(out=ident, in_=identF)
            rT = big.tile([P, KT, RB], F32R, name="rT")
            dacc = big.tile([P, ST * JB], F32, name="dacc")

            def inv_norm(x, scale, label):
                """[128,1] tile holding scale/||row|| for a [128, D] f32r tile.

                scale folds constants (like 1/T) in for free: Sqrt computes
                sqrt(ssq/scale^2) = ||row||/scale, reciprocal flips it.
                """
                sq = sqp.tile([P, D], F32, tag="sq", name="sq")
                ssq = vecp.tile([P, 1], F32, tag="ssq", name="ssq")
                nc.scalar.activation(
                    out=sq, in_=x.bitcast(F32), func=AF.Square, accum_out=ssq
                )
                nrm = vecp.tile([P, 1], F32, tag="nrm", name="nrm")
                nc.scalar.activation(
                    out=nrm, in_=ssq, func=AF.Sqrt, scale=1.0 / (scale * scale)
                )
                inv = vecp.tile([P, 1], F32R, tag="inv", name=f"inv_{label}")
                with nc.allow_low_precision(reason="f32r matmul operand"):
                    nc.vector.reciprocal(out=inv, in_=nrm)
                return inv

            def transpose_tile(x, dst):
                """PE-transpose a [128, D] f32r tile into dst [128, KT, 128]."""
                for kb in range(KT // 4):
                    tp = tpsum.tile([P, 512], F32R, tag="tp", name="tp")
                    for q in range(4):
                        k = kb * 4 + q
                        nc.tensor.transpose(
                            tp[:, q * P : (q + 1) * P],
                            x[:, k * P : (k + 1) * P],
                            ident,
                        )
                    nc.vector.tensor_copy(
                        out=dst[:, kb * 4 : (kb + 1) * 4, :],
                        in_=tp.rearrange("p (a b) -> p a b", a=4),
                    )

            def numerator_mms(x, inv, chunks, start, stop):
                for cc in range(4):
                    nc.tensor.matmul(
                        chunks[cc],
                        lhsT=inv,
                        rhs=x[:, cc * 512 : (cc + 1) * 512],
                        start=start,
                        stop=stop,
                    )

            def flush_chunks(chunks, out_dram, label):
                osb = vecp.tile([1, D], F32, tag="osb", name=f"osb_{label}")
                for cc in range(4):
                    nc.vector.tensor_copy(
                        out=osb[:, cc * 512 : (cc + 1) * 512], in_=chunks[cc]
                    )
                nc.sync.dma_start(out=out_dram[:, :], in_=osb)

            # ---- B shard: numerator partial only
            bchunks = [
                vpsum.tile([1, 512], F32, tag=f"vp{cc}", name=f"vpb{cc}")
                for cc in range(4)
            ]
            for t in range(NSH // P):
                bx = xin.tile([P, D], F32R, tag="xin", name="bx")
                nc.sync.dma_start(out=bx, in_=b_shard[t * P : (t + 1) * P, :])
                inv_b = inv_norm(bx, 1.0, "b")
                numerator_mms(bx, inv_b, bchunks, t == 0, t == NSH // P - 1)
            flush_chunks(bchunks, bsum, "b")

            # ---- R block: normalize rows (on DVE), transpose into rT
            for jt in range(RT):
                rx = xin.tile([P, D], F32R, tag="xin", name="rx")
                nc.sync.dma_start(out=rx, in_=r_block[jt * P : (jt + 1) * P, :])
                inv_r = inv_norm(rx, 1.0, "r")
                rxn = xin.tile([P, D], F32R, tag="xn", bufs=2, name="rxn")
                nc.vector.tensor_scalar_mul(
                    rxn, rx.bitcast(F32), inv_r.bitcast(F32)
                )
                transpose_tile(rxn, rT[:, :, jt * P : (jt + 1) * P])

            # ---- S loop: raw transpose, matmul vs rT, exp+reduce.
            # The numerator shard for core a*4+b is local tiles [b*4, b*4+4);
            # its matmuls use inv_sT = (1/T)/||row||, host rescales by T.
            schunks = [
                vpsum.tile([1, 512], F32, tag=f"vp{cc}", name=f"vps{cc}")
                for cc in range(4)
            ]
            num_lo = -1  # patched per-core? no: same program all cores -> use
            # core-independent trick: every core sums tiles [0, 4) of its own
            # S block and the host picks the right 512 rows by passing them
            # as the FIRST tiles. See _make_in_maps: s_block rows are rotated
            # so rows b*512..(b+1)*512 come first.
            num_lo = 0
            num_hi = 4
            for st in range(ST):
                sx = xin.tile([P, D], F32R, tag="xin", name="sx")
                nc.sync.dma_start(out=sx, in_=s_block[st * P : (st + 1) * P, :])
                inv_sT = inv_norm(sx, 1.0 / TEMP, "s")
                if num_lo <= st < num_hi:
                    numerator_mms(
                        sx, inv_sT, schunks, st == num_lo, st == num_hi - 1
                    )
                sT = stp.tile([P, KT, P], F32R, tag="sT", name="sT")
                transpose_tile(sx, sT)
                for jb in range(JB):
                    g = gpsum.tile([P, 512], F32, tag="g", name="g")
                    for k in range(KT):
                        nc.tensor.matmul(
                            g,
                            lhsT=sT[:, k, :],
                            rhs=rT[:, k, jb * 512 : (jb + 1) * 512],
                            start=(k == 0),
                            stop=(k == KT - 1),
                        )
                    esc = escp.tile([P, 512], F32, tag="esc", name="esc")
                    col = st * JB + jb
                    nc.scalar.activation(
                        out=esc,
                        in_=g,
                        func=AF.Exp,
                        scale=inv_sT.bitcast(F32),
                        accum_out=dacc[:, col : col + 1],
                    )
            flush_chunks(schunks, ssum, "s")

            nc.sync.dma_start(out=denom_acc[:, :], in_=dacc)
    return nc


_NC_CACHE = {}


def _get_nc():
    if "nc" not in _NC_CACHE:
        _NC_CACHE["nc"] = build()
    return _NC_CACHE["nc"]


def _make_in_maps(x_source, x_bc_target, x_raw_target):
    in_maps = []
    for c in range(8):
        a, b = c // B_SPLIT, c % B_SPLIT
        sblk = x_source[a * SB : (a + 1) * SB]
        # Rotate so the core's numerator shard (local rows b*512..(b+1)*512)
        # lands in tiles [0, 4) — the kernel always numerates its first 4.
        sblk = np.concatenate([sblk[b * NSH : (b + 1) * NSH], sblk[: b * NSH], sblk[(b + 1) * NSH :]], axis=0)
        in_maps.append(
            {
                "s_block": np.ascontiguousarray(sblk, dtype=np.float32),
                "r_block": np.ascontiguousarray(
                    x_raw_target[b * RB : (b + 1) * RB], dtype=np.float32
                ),
                "b_shard": np.ascontiguousarray(
                    x_bc_target[c * NSH : (c + 1) * NSH], dtype=np.float32
                ),
            }
        )
    return in_maps


def _combine(results):
    denom = 0.0
    s_tot = np.zeros(D, dtype=np.float64)
    b_tot = np.zeros(D, dtype=np.float64)
    for r in results:
        denom += r["denom_acc"].astype(np.float64).sum()
        s_tot += r["ssum"][0].astype(np.float64)
        b_tot += r["bsum"][0].astype(np.float64)
    s_tot *= TEMP  # undo the 1/T fold in inv_sT
    loss = np.log(denom) - (s_tot @ b_tot) / (float(N) * float(N)) / TEMP
    return np.array(loss, dtype=np.float32)


def _run(x_source, x_bc_target, x_raw_target, trace=False):
    nc = _get_nc()
    in_maps = _make_in_maps(x_source, x_bc_target, x_raw_target)
    res = run_bass_kernel_spmd(nc, in_maps, core_ids=list(range(8)), trace=trace)
    return _combine(res.results), res


def kernel(x_source, x_bc_target, x_raw_target):
    out, _ = _run(x_source, x_bc_target, x_raw_target)
    return out
